# revision 11
# baseline (speedup 1.0000x reference)
"""Trainium2 Bass kernel for nn_BertEncoder_403726926494.

Reference computation (per batch element):
  - ragged sentence extraction from hidden_states, masked-softmax attention
    pooling per sentence with W_doc            -> doc_pooled [B, D, H]
  - query extraction (rows 1..32), masked-softmax pooling with W_query
    broadcast over D                           -> q_bcast   [B, D, H]

Device strategy (SPMD, one program on 8 cores, 8 batch elements per core):
  - Dense token packing: each core receives ONE fp16 token stream = the
    concatenation of its 8 examples' doc-sentence tokens, padded to a 128
    boundary, followed by a copy of their query tokens.  Token t lands on
    SBUF partition t%128 of chunk t//128.  A trailing ones-column (col 768)
    rides along for the softmax denominators.
  - Per chunk: ONE fused DVE scalar_tensor_tensor computes xw = x*W and
    accumulates the per-token score s_t (fp32) in the same pass; ONE ACT
    Exp over a host-built log-mask (0 where token t belongs to pooling
    column m, -4096 elsewhere) with bias=s_t yields
    at[t,m] = exp(s_t)*onehot[t,m].
  - Doc pooling columns: 8 examples x 16 sentences = exactly 128 PE columns,
    so ONE PSUM accumulation group [128, 769] collects num|den for every
    sentence of the core across all doc chunks (fp16 matmuls, fp32 PSUM).
    Query chunks accumulate into a second small group [32, 769] (8 cols used).
  - out = num * (1/(den+eps)); results leave as fp16 and are scattered back
    to [B, D, H] on the host.  b_doc / b_query shift every score in a softmax
    segment equally, so they cancel and are ignored.
"""

import numpy as np
import ml_dtypes

B, L, H = 64, 512, 768
D, S, Q = 16, 64, 32
NCORES = 8
EPB = 8  # examples per core
NEG = -4096.0  # exp(NEG + s) == 0
DEN_EPS = 1.0e-30

F16 = np.float16
MASK_F8 = True  # log-mask as fp8e5 (halves mask DMA); flip to False if flaky
F8 = ml_dtypes.float8_e5m2 if MASK_F8 else np.float16

# Score-pass engine plan (tuning knobs). Per chunk:
#   "stt"  - one fused DVE scalar_tensor_tensor (1x rate, one op)
#   "act"  - DVE tensor_tensor (2x rate) + ACT Copy-accum reduce
#   "gps"  - DVE tensor_tensor (2x rate) + GpSimd tensor_reduce
#   "gtt"  - GpSimd tensor_tensor + ACT Copy-accum reduce
#   "dtt"  - DVE tensor_tensor (2x rate) + DVE tensor_reduce
SCORE_PLAN = {1: "gtt", 5: "gtt", 9: "gtt", 11: "gtt", 3: "dtt", 7: "dtt"}


def _score_mode(c):
    return SCORE_PLAN.get(c, "stt")

_compiled: dict = {}


def _build(NTD, NTQ, REM_D, REM_Q):
    """Build + compile the SPMD Bass program for the given chunk geometry."""
    from contextlib import ExitStack

    import concourse.bacc as bacc
    import concourse.tile as tile
    from concourse import mybir

    f32 = mybir.dt.float32
    f16 = mybir.dt.float16
    f8 = mybir.dt.float8e5 if MASK_F8 else mybir.dt.float16
    MULT = mybir.AluOpType.mult
    ADD = mybir.AluOpType.add
    EXP = mybir.ActivationFunctionType.Exp
    COPY = mybir.ActivationFunctionType.Copy

    NT = NTD + NTQ
    W = H + 1  # 769: H data cols + ones col

    nc = bacc.Bacc(
        "TRN2", target_bir_lowering=False, debug=False, num_devices=NCORES
    )
    xdf = nc.dram_tensor("xdf", [128, max(NTD - 1, 1), W], f16, kind="ExternalInput").ap()
    xdr = nc.dram_tensor("xdr", [REM_D, W], f16, kind="ExternalInput").ap()
    xqf = nc.dram_tensor("xqf", [128, max(NTQ - 1, 1), W], f16, kind="ExternalInput").ap()
    xqr = nc.dram_tensor("xqr", [REM_Q, W], f16, kind="ExternalInput").ap()
    mask8 = nc.dram_tensor("mask8", [128, NT, 128], f8, kind="ExternalInput").ap()
    wbd = nc.dram_tensor("wbd", [128, H], f16, kind="ExternalInput").ap()
    wbq = nc.dram_tensor("wbq", [128, H], f16, kind="ExternalInput").ap()
    doc_out = nc.dram_tensor("doc_out", [128, H], f16, kind="ExternalOutput").ap()
    q_out = nc.dram_tensor("q_out", [EPB, H], f16, kind="ExternalOutput").ap()

    with tile.TileContext(nc) as tc, ExitStack() as ctx:
        const = ctx.enter_context(tc.tile_pool(name="const", bufs=1))
        atp = ctx.enter_context(tc.tile_pool(name="atp", bufs=3))
        scrp = ctx.enter_context(tc.tile_pool(name="scr", bufs=4))
        outp = ctx.enter_context(tc.tile_pool(name="outp", bufs=2))
        smallp = ctx.enter_context(tc.tile_pool(name="smallp", bufs=4))
        nump = ctx.enter_context(tc.tile_pool(name="nump", bufs=1, space="PSUM"))
        qnump = ctx.enter_context(tc.tile_pool(name="qnump", bufs=1, space="PSUM"))

        x = const.tile([128, NT, W], f16)
        mask_t = const.tile([128, NT, 128], f8)
        wb_d = const.tile([128, H], f16)
        wb_q = const.tile([128, H], f16)
        scol = const.tile([128, NT], f32)
        s2 = const.tile([128, H], f16)  # dummy out for ACT accum reduce

        # zero the partial chunks up front so padding rows stay finite
        # (mask NEG keeps them out of the pools; engine partition access
        # must start at 0, so clear whole chunks, then partial-DMA)
        if REM_D < 128:
            nc.vector.memset(x[:, NTD - 1, :], 0.0)
        if REM_Q < 128:
            nc.vector.memset(x[:, NT - 1, :], 0.0)

        # ---- input DMAs, split across both HWDGE rings so descriptor
        # issue parallelizes: sync ring carries the big doc-x stream,
        # scalar ring carries weights/mask/query-x (+ small outputs)
        nc.scalar.dma_start(out=wb_d[:], in_=wbd[:])
        nc.scalar.dma_start(out=mask_t[:], in_=mask8[:])
        nfd = NTD - 1
        bounds = sorted({min(g, nfd) for g in (1, 2, 4, 7, nfd)} | {0})
        for a, b in zip(bounds[:-1], bounds[1:]):
            if b > a:
                nc.sync.dma_start(out=x[:, a:b, :], in_=xdf[:, a:b, :])
        nc.sync.dma_start(out=x[0:REM_D, NTD - 1, :], in_=xdr[:])
        nc.scalar.dma_start(out=wb_q[:], in_=wbq[:])
        if NTQ > 1:
            nc.scalar.dma_start(out=x[:, NTD : NT - 1, :], in_=xqf[:])
        nc.scalar.dma_start(out=x[0:REM_Q, NT - 1, :], in_=xqr[:])

        numg = nump.tile([128, 1024], f32, tag="num", name="num")
        qnumg = qnump.tile([32, 1024], f32, tag="qnum", name="qnum")

        def emit_chunk(c):
            wb = wb_d if c < NTD else wb_q
            mcols = 128 if c < NTD else 32
            psum = numg if c < NTD else qnumg
            start = c == 0 or c == NTD
            stop = c == NTD - 1 or c == NT - 1

            xw = scrp.tile([128, H], f16, tag="xw", name=f"xw{c}")
            mode = _score_mode(c)
            if mode == "stt":
                nc.vector.scalar_tensor_tensor(
                    out=xw[:], in0=x[:, c, 0:H], scalar=1.0, in1=wb[:],
                    op0=MULT, op1=MULT, accum_out=scol[:, c : c + 1],
                )
            elif mode == "gtt":
                nc.gpsimd.tensor_tensor(
                    out=xw[:], in0=x[:, c, 0:H], in1=wb[:], op=MULT
                )
                nc.scalar.activation(
                    s2[:], xw[:], COPY, bias=0.0, scale=1.0,
                    accum_out=scol[:, c : c + 1],
                )
            else:
                nc.vector.tensor_tensor(
                    out=xw[:], in0=x[:, c, 0:H], in1=wb[:], op=MULT
                )
                nc.vector.tensor_reduce(
                    out=scol[:, c : c + 1], in_=xw[:],
                    axis=mybir.AxisListType.X, op=ADD,
                )
            at = atp.tile([128, 128], f16, tag="at", name=f"at{c}")
            nc.scalar.activation(
                at[:, 0:mcols], mask_t[:, c, 0:mcols], EXP,
                bias=scol[:, c : c + 1], scale=1.0,
            )
            nc.tensor.matmul(
                psum[0:mcols, 0:512], at[:, 0:mcols], x[:, c, 0:512],
                start=start, stop=stop,
            )
            nc.tensor.matmul(
                psum[0:mcols, 512:W], at[:, 0:mcols], x[:, c, 512:W],
                start=start, stop=stop,
            )

        for c in range(NT):
            emit_chunk(c)

        # ---- doc finish: out = num / (den + eps) ----
        de = smallp.tile([128, 1], f32, tag="de", name="de")
        nc.vector.tensor_scalar(
            out=de[:], in0=numg[:, H : H + 1], scalar1=DEN_EPS, scalar2=None, op0=ADD
        )
        rec = smallp.tile([128, 1], f32, tag="rec", name="rec")
        nc.vector.reciprocal(rec[:], de[:])
        do = outp.tile([128, H], f16, tag="do", name="do")
        nc.scalar.activation(do[:], numg[:, 0:H], COPY, bias=0.0, scale=rec[:, 0:1])
        nc.sync.dma_start(out=doc_out[:], in_=do[:])

        # ---- query finish ----
        qde = smallp.tile([EPB, 1], f32, tag="qde", name="qde")
        nc.vector.tensor_scalar(
            out=qde[:], in0=qnumg[0:EPB, H : H + 1], scalar1=DEN_EPS,
            scalar2=None, op0=ADD,
        )
        qrec = smallp.tile([EPB, 1], f32, tag="qrec", name="qrec")
        nc.vector.reciprocal(qrec[:], qde[:])
        qo = outp.tile([EPB, H], f16, tag="qo", name="qo")
        nc.scalar.activation(
            qo[:], qnumg[0:EPB, 0:H], COPY, bias=0.0, scale=qrec[:, 0:1]
        )
        nc.scalar.dma_start(out=q_out[:], in_=qo[:])

    nc.compile()
    return nc


def _prepare(query_len, seq_lens):
    """Host-side geometry: example->core assignment + per-core streams."""
    ql = np.asarray(query_len).astype(np.int64)
    sl = np.asarray(seq_lens).astype(np.int64)
    offs = ql[:, None] + 2 + np.cumsum(sl, axis=1) - sl  # [B, D] sentence starts
    doc_tok = sl.sum(axis=1)

    # greedy balance of total tokens into NCORES bins of EPB examples
    tot = doc_tok + ql
    order = np.argsort(-tot, kind="stable")
    ex_map = [[] for _ in range(NCORES)]
    loads = np.zeros(NCORES, np.int64)
    for e in order:
        cand = [c for c in range(NCORES) if len(ex_map[c]) < EPB]
        c = min(cand, key=lambda cc: (loads[cc], cc))
        ex_map[c].append(int(e))
        loads[c] += tot[e]

    drows = np.array([sum(doc_tok[e] for e in ex_map[c]) for c in range(NCORES)])
    qrows = np.array([sum(ql[e] for e in ex_map[c]) for c in range(NCORES)])
    NTD = int(max(-(-r // 128) for r in drows))
    NTQ = int(max(-(-r // 128) for r in qrows))
    REM_D = max(int(max(r - 128 * (NTD - 1) for r in drows)), 1)
    REM_Q = max(int(max(r - 128 * (NTQ - 1) for r in qrows)), 1)

    # per-core gather indices (into hs.reshape(B*L, H)) and pooling col ids
    plan = []
    for c in range(NCORES):
        didx, dsid, qidx, qsid = [], [], [], []
        for k, e in enumerate(ex_map[c]):
            for j in range(D):
                n = int(sl[e, j])
                if n == 0:
                    continue
                o = int(offs[e, j])
                didx.append(np.arange(e * L + o, e * L + o + n))
                dsid.append(np.full(n, 16 * k + j))
            n = int(ql[e])
            qidx.append(np.arange(e * L + 1, e * L + 1 + n))
            qsid.append(np.full(n, k))
        plan.append(
            (
                np.concatenate(didx),
                np.concatenate(dsid),
                np.concatenate(qidx),
                np.concatenate(qsid),
            )
        )
    return ex_map, plan, (NTD, NTQ, REM_D, REM_Q)


def _stage_core(hs2, plan_c, geom):
    NTD, NTQ, REM_D, REM_Q = geom
    NT = NTD + NTQ
    W = H + 1
    didx, dsid, qidx, qsid = plan_c
    nd_pad = 128 * (NTD - 1) + REM_D
    nq_pad = 128 * (NTQ - 1) + REM_Q

    def pack(idx, npad):
        xs = np.zeros((npad, W), F16)
        xs[: len(idx), 0:H] = hs2[idx]
        xs[: len(idx), H] = 1.0
        return xs

    xd = pack(didx, nd_pad)
    xq = pack(qidx, nq_pad)
    xdf = np.ascontiguousarray(
        xd[: 128 * (NTD - 1)].reshape(max(NTD - 1, 1), -1, W)[:, :128].transpose(1, 0, 2)
        if NTD > 1
        else np.zeros((128, 1, W), F16)
    )
    xdr = np.ascontiguousarray(xd[128 * (NTD - 1) :])
    xqf = np.ascontiguousarray(
        xq[: 128 * (NTQ - 1)].reshape(max(NTQ - 1, 1), -1, W)[:, :128].transpose(1, 0, 2)
        if NTQ > 1
        else np.zeros((128, 1, W), F16)
    )
    xqr = np.ascontiguousarray(xq[128 * (NTQ - 1) :])

    mlin = np.full((NT * 128, 128), NEG, np.float32)
    mlin[np.arange(len(dsid)), dsid] = 0.0
    qbase = NTD * 128
    mlin[qbase + np.arange(len(qsid)), qsid] = 0.0
    mask8 = np.ascontiguousarray(
        mlin.reshape(NT, 128, 128).transpose(1, 0, 2)
    ).astype(F8)
    return xdf, xdr, xqf, xqr, mask8


def kernel(hidden_states, W_doc, b_doc, W_query, b_query, query_len, seq_lens):
    hs = np.asarray(hidden_states, dtype=np.float32)
    hs2 = hs.reshape(B * L, H)
    wd = np.ascontiguousarray(
        np.broadcast_to(
            np.asarray(W_doc, np.float32).reshape(1, H).astype(F16), (128, H)
        )
    )
    wq = np.ascontiguousarray(
        np.broadcast_to(
            np.asarray(W_query, np.float32).reshape(1, H).astype(F16), (128, H)
        )
    )

    ex_map, plan, geom = _prepare(query_len, seq_lens)

    nc = _compiled.get(geom)
    if nc is None:
        nc = _build(*geom)
        _compiled[geom] = nc

    in_maps = []
    for c in range(NCORES):
        xdf, xdr, xqf, xqr, mask8 = _stage_core(hs2, plan[c], geom)
        in_maps.append(
            {
                "xdf": xdf,
                "xdr": xdr,
                "xqf": xqf,
                "xqr": xqr,
                "mask8": mask8,
                "wbd": wd,
                "wbq": wq,
            }
        )

    from concourse.bass_utils import run_bass_kernel_spmd

    res = run_bass_kernel_spmd(nc, in_maps, list(range(NCORES)))

    doc = np.empty((B, D, H), np.float32)
    qp = np.empty((B, H), np.float32)
    for c in range(NCORES):
        r = res.results[c]
        dall = np.asarray(r["doc_out"], np.float32).reshape(EPB, D, H)
        qall = np.asarray(r["q_out"], np.float32)
        for k, e in enumerate(ex_map[c]):
            doc[e] = dall[k]
            qp[e] = qall[k]
    q_bcast = np.broadcast_to(qp[:, None, :], (B, D, H))
    return doc, q_bcast


# revision 12
# speedup vs baseline: 1.1397x; 1.1397x over previous
"""Trainium2 Bass kernel for nn_BertEncoder_403726926494.

Reference computation (per batch element):
  - ragged sentence extraction from hidden_states, masked-softmax attention
    pooling per sentence with W_doc            -> doc_pooled [B, D, H]
  - query extraction (rows 1..32), masked-softmax pooling with W_query
    broadcast over D                           -> q_bcast   [B, D, H]

Device strategy (SPMD, one program on 8 cores, 8 batch elements per core):
  - Dense token packing: each core receives ONE fp16 token stream = the
    concatenation of its 8 examples' doc-sentence tokens, padded to a 128
    boundary, followed by a copy of their query tokens.  Token t lands on
    SBUF partition t%128 of chunk t//128.  A trailing ones-column (col 768)
    rides along for the softmax denominators.
  - Per chunk: ONE fused DVE scalar_tensor_tensor computes xw = x*W and
    accumulates the per-token score s_t (fp32) in the same pass; ONE ACT
    Exp over a host-built log-mask (0 where token t belongs to pooling
    column m, -4096 elsewhere) with bias=s_t yields
    at[t,m] = exp(s_t)*onehot[t,m].
  - Doc pooling columns: 8 examples x 16 sentences = exactly 128 PE columns,
    so ONE PSUM accumulation group [128, 769] collects num|den for every
    sentence of the core across all doc chunks (fp16 matmuls, fp32 PSUM).
    Query chunks accumulate into a second small group [32, 769] (8 cols used).
  - out = num * (1/(den+eps)); results leave as fp16 and are scattered back
    to [B, D, H] on the host.  b_doc / b_query shift every score in a softmax
    segment equally, so they cancel and are ignored.
"""

import numpy as np
import ml_dtypes

B, L, H = 64, 512, 768
D, S, Q = 16, 64, 32
NCORES = 8
EPB = 8  # examples per core
NEG = -4096.0  # exp(NEG + s) == 0
DEN_EPS = 1.0e-30

F16 = np.float16
MASK_F8 = True  # log-mask as fp8e5 (halves mask DMA); flip to False if flaky
F8 = ml_dtypes.float8_e5m2 if MASK_F8 else np.float16

# Score-pass engine plan (tuning knobs). Per chunk:
#   "stt"  - one fused DVE scalar_tensor_tensor (1x rate, one op)
#   "act"  - DVE tensor_tensor (2x rate) + ACT Copy-accum reduce
#   "gps"  - DVE tensor_tensor (2x rate) + GpSimd tensor_reduce
#   "gtt"  - GpSimd tensor_tensor + ACT Copy-accum reduce
#   "dtt"  - DVE tensor_tensor (2x rate) + DVE tensor_reduce
SCORE_PLAN: dict = {}  # uniform STT on DVE; mixing op types on DVE is slow


def _score_mode(c):
    return SCORE_PLAN.get(c, "stt")

_compiled: dict = {}


def _build(NTD, NTQ, REM_D, REM_Q):
    """Build + compile the SPMD Bass program for the given chunk geometry."""
    from contextlib import ExitStack

    import concourse.bacc as bacc
    import concourse.tile as tile
    from concourse import mybir

    f32 = mybir.dt.float32
    f16 = mybir.dt.float16
    f8 = mybir.dt.float8e5 if MASK_F8 else mybir.dt.float16
    MULT = mybir.AluOpType.mult
    ADD = mybir.AluOpType.add
    EXP = mybir.ActivationFunctionType.Exp
    COPY = mybir.ActivationFunctionType.Copy

    NT = NTD + NTQ
    W = H + 1  # 769: H data cols + ones col

    nc = bacc.Bacc(
        "TRN2", target_bir_lowering=False, debug=False, num_devices=NCORES
    )
    xdf = nc.dram_tensor("xdf", [128, max(NTD - 1, 1), W], f16, kind="ExternalInput").ap()
    xdr = nc.dram_tensor("xdr", [REM_D, W], f16, kind="ExternalInput").ap()
    xqf = nc.dram_tensor("xqf", [128, max(NTQ - 1, 1), W], f16, kind="ExternalInput").ap()
    xqr = nc.dram_tensor("xqr", [REM_Q, W], f16, kind="ExternalInput").ap()
    mask8 = nc.dram_tensor("mask8", [128, NT, 128], f8, kind="ExternalInput").ap()
    wbd = nc.dram_tensor("wbd", [128, H], f16, kind="ExternalInput").ap()
    wbq = nc.dram_tensor("wbq", [128, H], f16, kind="ExternalInput").ap()
    doc_out = nc.dram_tensor("doc_out", [128, H], f16, kind="ExternalOutput").ap()
    q_out = nc.dram_tensor("q_out", [EPB, H], f16, kind="ExternalOutput").ap()

    with tile.TileContext(nc) as tc, ExitStack() as ctx:
        const = ctx.enter_context(tc.tile_pool(name="const", bufs=1))
        atp = ctx.enter_context(tc.tile_pool(name="atp", bufs=3))
        scrp = ctx.enter_context(tc.tile_pool(name="scr", bufs=4))
        outp = ctx.enter_context(tc.tile_pool(name="outp", bufs=2))
        smallp = ctx.enter_context(tc.tile_pool(name="smallp", bufs=4))
        nump = ctx.enter_context(tc.tile_pool(name="nump", bufs=1, space="PSUM"))
        qnump = ctx.enter_context(tc.tile_pool(name="qnump", bufs=1, space="PSUM"))

        x = const.tile([128, NT, W], f16)
        mask_t = const.tile([128, NT, 128], f8)
        wb_d = const.tile([128, H], f16)
        wb_q = const.tile([128, H], f16)
        scol = const.tile([128, NT], f32)
        s2 = const.tile([128, H], f16)  # dummy out for ACT accum reduce

        # zero the partial chunks up front so padding rows stay finite
        # (mask NEG keeps them out of the pools; engine partition access
        # must start at 0, so clear whole chunks, then partial-DMA)
        if REM_D < 128:
            nc.gpsimd.memset(x[:, NTD - 1, :], 0.0)
        if REM_Q < 128:
            nc.gpsimd.memset(x[:, NT - 1, :], 0.0)

        # ---- input DMAs, split across both HWDGE rings so descriptor
        # issue parallelizes: sync ring carries the big doc-x stream,
        # scalar ring carries weights/mask/query-x (+ small outputs)
        nc.scalar.dma_start(out=wb_d[:], in_=wbd[:])
        nc.scalar.dma_start(out=mask_t[:], in_=mask8[:])
        nfd = NTD - 1
        bounds = sorted({min(g, nfd) for g in (1, 2, 4, 7, nfd)} | {0})
        for a, b in zip(bounds[:-1], bounds[1:]):
            if b > a:
                nc.sync.dma_start(out=x[:, a:b, :], in_=xdf[:, a:b, :])
        nc.sync.dma_start(out=x[0:REM_D, NTD - 1, :], in_=xdr[:])
        nc.scalar.dma_start(out=wb_q[:], in_=wbq[:])
        if NTQ > 1:
            nc.sync.dma_start(out=x[:, NTD : NT - 1, :], in_=xqf[:])
        nc.sync.dma_start(out=x[0:REM_Q, NT - 1, :], in_=xqr[:])

        numg = nump.tile([128, 1024], f32, tag="num", name="num")
        qnumg = qnump.tile([32, 1024], f32, tag="qnum", name="qnum")

        def emit_chunk(c):
            wb = wb_d if c < NTD else wb_q
            mcols = 128 if c < NTD else 32
            psum = numg if c < NTD else qnumg
            start = c == 0 or c == NTD
            stop = c == NTD - 1 or c == NT - 1

            xw = scrp.tile([128, H], f16, tag="xw", name=f"xw{c}")
            mode = _score_mode(c)
            if mode == "stt":
                nc.vector.scalar_tensor_tensor(
                    out=xw[:], in0=x[:, c, 0:H], scalar=1.0, in1=wb[:],
                    op0=MULT, op1=MULT, accum_out=scol[:, c : c + 1],
                )
            elif mode == "gtt":
                nc.gpsimd.tensor_tensor(
                    out=xw[:], in0=x[:, c, 0:H], in1=wb[:], op=MULT
                )
                nc.scalar.activation(
                    s2[:], xw[:], COPY, bias=0.0, scale=1.0,
                    accum_out=scol[:, c : c + 1],
                )
            else:
                nc.vector.tensor_tensor(
                    out=xw[:], in0=x[:, c, 0:H], in1=wb[:], op=MULT
                )
                nc.vector.tensor_reduce(
                    out=scol[:, c : c + 1], in_=xw[:],
                    axis=mybir.AxisListType.X, op=ADD,
                )
            at = atp.tile([128, 128], f16, tag="at", name=f"at{c}")
            nc.scalar.activation(
                at[:, 0:mcols], mask_t[:, c, 0:mcols], EXP,
                bias=scol[:, c : c + 1], scale=1.0,
            )
            nc.tensor.matmul(
                psum[0:mcols, 0:512], at[:, 0:mcols], x[:, c, 0:512],
                start=start, stop=stop,
            )
            nc.tensor.matmul(
                psum[0:mcols, 512:W], at[:, 0:mcols], x[:, c, 512:W],
                start=start, stop=stop,
            )

        for c in range(NTD):
            emit_chunk(c)

        # ---- doc finish: out = num / (den + eps); overlaps query chunks ----
        de = smallp.tile([128, 1], f32, tag="de", name="de")
        nc.vector.tensor_scalar(
            out=de[:], in0=numg[:, H : H + 1], scalar1=DEN_EPS, scalar2=None, op0=ADD
        )
        rec = smallp.tile([128, 1], f32, tag="rec", name="rec")
        nc.vector.reciprocal(rec[:], de[:])
        do = outp.tile([128, H], f16, tag="do", name="do")
        nc.scalar.activation(do[:], numg[:, 0:H], COPY, bias=0.0, scale=rec[:, 0:1])
        nc.sync.dma_start(out=doc_out[:], in_=do[:])

        for c in range(NTD, NT):
            emit_chunk(c)

        # ---- query finish ----
        qde = smallp.tile([EPB, 1], f32, tag="qde", name="qde")
        nc.vector.tensor_scalar(
            out=qde[:], in0=qnumg[0:EPB, H : H + 1], scalar1=DEN_EPS,
            scalar2=None, op0=ADD,
        )
        qrec = smallp.tile([EPB, 1], f32, tag="qrec", name="qrec")
        nc.vector.reciprocal(qrec[:], qde[:])
        qo = outp.tile([EPB, H], f16, tag="qo", name="qo")
        nc.scalar.activation(
            qo[:], qnumg[0:EPB, 0:H], COPY, bias=0.0, scale=qrec[:, 0:1]
        )
        nc.scalar.dma_start(out=q_out[:], in_=qo[:])

    nc.compile()
    return nc


def _prepare(query_len, seq_lens):
    """Host-side geometry: example->core assignment + per-core streams."""
    ql = np.asarray(query_len).astype(np.int64)
    sl = np.asarray(seq_lens).astype(np.int64)
    offs = ql[:, None] + 2 + np.cumsum(sl, axis=1) - sl  # [B, D] sentence starts
    doc_tok = sl.sum(axis=1)

    # greedy balance of total tokens into NCORES bins of EPB examples
    tot = doc_tok + ql
    order = np.argsort(-tot, kind="stable")
    ex_map = [[] for _ in range(NCORES)]
    loads = np.zeros(NCORES, np.int64)
    for e in order:
        cand = [c for c in range(NCORES) if len(ex_map[c]) < EPB]
        c = min(cand, key=lambda cc: (loads[cc], cc))
        ex_map[c].append(int(e))
        loads[c] += tot[e]

    drows = np.array([sum(doc_tok[e] for e in ex_map[c]) for c in range(NCORES)])
    qrows = np.array([sum(ql[e] for e in ex_map[c]) for c in range(NCORES)])
    NTD = int(max(-(-r // 128) for r in drows))
    NTQ = int(max(-(-r // 128) for r in qrows))
    REM_D = max(int(max(r - 128 * (NTD - 1) for r in drows)), 1)
    REM_Q = max(int(max(r - 128 * (NTQ - 1) for r in qrows)), 1)

    # per-core gather indices (into hs.reshape(B*L, H)) and pooling col ids
    plan = []
    for c in range(NCORES):
        didx, dsid, qidx, qsid = [], [], [], []
        for k, e in enumerate(ex_map[c]):
            for j in range(D):
                n = int(sl[e, j])
                if n == 0:
                    continue
                o = int(offs[e, j])
                didx.append(np.arange(e * L + o, e * L + o + n))
                dsid.append(np.full(n, 16 * k + j))
            n = int(ql[e])
            qidx.append(np.arange(e * L + 1, e * L + 1 + n))
            qsid.append(np.full(n, k))
        plan.append(
            (
                np.concatenate(didx),
                np.concatenate(dsid),
                np.concatenate(qidx),
                np.concatenate(qsid),
            )
        )
    return ex_map, plan, (NTD, NTQ, REM_D, REM_Q)


def _stage_core(hs2, plan_c, geom):
    NTD, NTQ, REM_D, REM_Q = geom
    NT = NTD + NTQ
    W = H + 1
    didx, dsid, qidx, qsid = plan_c
    nd_pad = 128 * (NTD - 1) + REM_D
    nq_pad = 128 * (NTQ - 1) + REM_Q

    def pack(idx, npad):
        xs = np.zeros((npad, W), F16)
        xs[: len(idx), 0:H] = hs2[idx]
        xs[: len(idx), H] = 1.0
        return xs

    xd = pack(didx, nd_pad)
    xq = pack(qidx, nq_pad)
    xdf = np.ascontiguousarray(
        xd[: 128 * (NTD - 1)].reshape(max(NTD - 1, 1), -1, W)[:, :128].transpose(1, 0, 2)
        if NTD > 1
        else np.zeros((128, 1, W), F16)
    )
    xdr = np.ascontiguousarray(xd[128 * (NTD - 1) :])
    xqf = np.ascontiguousarray(
        xq[: 128 * (NTQ - 1)].reshape(max(NTQ - 1, 1), -1, W)[:, :128].transpose(1, 0, 2)
        if NTQ > 1
        else np.zeros((128, 1, W), F16)
    )
    xqr = np.ascontiguousarray(xq[128 * (NTQ - 1) :])

    mlin = np.full((NT * 128, 128), NEG, np.float32)
    mlin[np.arange(len(dsid)), dsid] = 0.0
    qbase = NTD * 128
    mlin[qbase + np.arange(len(qsid)), qsid] = 0.0
    mask8 = np.ascontiguousarray(
        mlin.reshape(NT, 128, 128).transpose(1, 0, 2)
    ).astype(F8)
    return xdf, xdr, xqf, xqr, mask8


def kernel(hidden_states, W_doc, b_doc, W_query, b_query, query_len, seq_lens):
    hs = np.asarray(hidden_states, dtype=np.float32)
    hs2 = hs.reshape(B * L, H)
    wd = np.ascontiguousarray(
        np.broadcast_to(
            np.asarray(W_doc, np.float32).reshape(1, H).astype(F16), (128, H)
        )
    )
    wq = np.ascontiguousarray(
        np.broadcast_to(
            np.asarray(W_query, np.float32).reshape(1, H).astype(F16), (128, H)
        )
    )

    ex_map, plan, geom = _prepare(query_len, seq_lens)

    nc = _compiled.get(geom)
    if nc is None:
        nc = _build(*geom)
        _compiled[geom] = nc

    in_maps = []
    for c in range(NCORES):
        xdf, xdr, xqf, xqr, mask8 = _stage_core(hs2, plan[c], geom)
        in_maps.append(
            {
                "xdf": xdf,
                "xdr": xdr,
                "xqf": xqf,
                "xqr": xqr,
                "mask8": mask8,
                "wbd": wd,
                "wbq": wq,
            }
        )

    from concourse.bass_utils import run_bass_kernel_spmd

    res = run_bass_kernel_spmd(nc, in_maps, list(range(NCORES)))

    doc = np.empty((B, D, H), np.float32)
    qp = np.empty((B, H), np.float32)
    for c in range(NCORES):
        r = res.results[c]
        dall = np.asarray(r["doc_out"], np.float32).reshape(EPB, D, H)
        qall = np.asarray(r["q_out"], np.float32)
        for k, e in enumerate(ex_map[c]):
            doc[e] = dall[k]
            qp[e] = qall[k]
    q_bcast = np.broadcast_to(qp[:, None, :], (B, D, H))
    return doc, q_bcast
